# revision 8
# baseline (speedup 1.0000x reference)
"""MoE layer (top-2 of 8 experts) on 8 TRN2 NeuronCores.

Strategy:
  Host: gate logits + softmax + top-2 + renormalized weights (the
      routing / sharding decision), build per-expert token index lists,
      pad to a common capacity C (multiple of the token block).
  Device (expert-parallel): core e runs its expert's FFN over
      the tokens routed to it: y = (relu(x@W1+b1)@W2 + b2) * w_token.
      bf16 matmuls, fp32 PSUM accumulation, weights SBUF-resident.
  Host: scatter-add the two scaled contributions per token.
"""

import numpy as np
import ml_dtypes

import concourse.mybir as mybir
import concourse.tile as tile
from concourse import bacc
from concourse.bass_utils import run_bass_kernel_spmd

P = 128
N_CORES = 8
CB = 384  # phase-2 token block
BF16 = mybir.dt.bfloat16
F32 = mybir.dt.float32
_bf16_np = ml_dtypes.bfloat16

_build_cache = {}


def _build_expert(D, H, O, C):
    """Per-core expert FFN over C (padded) routed tokens.

    y[C, O] = (relu(x @ W1 + b1) @ W2 + b2) * w_token[:, None]
    computed as hT = W1.T-slices @ xT (keeps H on partitions), then
    y = hT-slices.T @ W2 (tokens back on partitions). No transposes on
    device: xT / b1 / wt come host-prearranged.
    """
    nc = bacc.Bacc(None, target_bir_lowering=False)
    xT = nc.dram_tensor("xT", [D, C], BF16, kind="ExternalInput")
    w1 = nc.dram_tensor("w1", [D, H], BF16, kind="ExternalInput")
    w2 = nc.dram_tensor("w2", [H, O], BF16, kind="ExternalInput")
    b1 = nc.dram_tensor("b1", [P, H // P], F32, kind="ExternalInput")
    b2 = nc.dram_tensor("b2", [P, O], F32, kind="ExternalInput")
    wt = nc.dram_tensor("wt", [P, C // P], F32, kind="ExternalInput")
    y = nc.dram_tensor("y", [C, O], F32, kind="ExternalOutput")
    DO, HO = D // P, H // P
    OO = O // 512
    # token blocks of CB, trailing 128-block if C % CB != 0
    starts = []
    pos = 0
    while pos < C:
        cb = CB if C - pos >= CB else C - pos
        starts.append((pos, cb))
        pos += cb
    # chunk the weight loads so the first matmuls start after ~1MB of DMA;
    # the first two W1 chunks are half-size so L1 starts even earlier
    HC = 4                   # h-tiles (of 128) per W2 weight chunk
    NWC = HO // HC           # number of W2 weight chunks
    w1_chunks = [(0, 1), (1, 1), (2, 2)] + [(h, 4) for h in range(4, HO, 4)]
    w1_of_hi = {}            # hi -> (chunk index, offset within chunk)
    for ci, (h0, nh) in enumerate(w1_chunks):
        for j in range(nh):
            w1_of_hi[h0 + j] = (ci, j)
    HG = 8                   # h-tiles per hT group tile (finer L2 deps)
    NHG = HO // HG
    y_r = y.rearrange("(n p) o -> p n o", p=P)
    w1_r = w1.rearrange("(do p) h -> p do h", p=P)
    w2_r = w2.rearrange("(ho p) o -> p ho o", p=P)
    with tile.TileContext(nc) as tc:
        with (
            tc.tile_pool(name="wpool", bufs=1) as wp,
            tc.tile_pool(name="xpool", bufs=3) as xp,
            tc.tile_pool(name="hpool", bufs=2) as hp,
            tc.tile_pool(name="opool", bufs=4) as op,
            tc.tile_pool(name="hps", bufs=4, space="PSUM") as hps,
            tc.tile_pool(name="yps", bufs=3, space="PSUM") as yps,
        ):
            xT_r = xT.rearrange("(do p) c -> p do c", p=P)
            # startup-critical DMAs: W1 chunks split across the sync AND
            # scalar rings (doubles early W1 bandwidth; L1 of block 0
            # chases W1's delivery); block-0 x leads on sync, b1 on
            # scalar, W2/b2/wt and later x blocks ride the gpsimd ring.
            x0_sb = xp.tile([P, DO, CB], BF16, tag="x")
            for dj in range(0, DO, 2):
                nc.scalar.dma_start(
                    x0_sb[:, dj:dj + 2, :starts[0][1]],
                    xT_r[:, dj:dj + 2, 0:starts[0][1]],
                )
            w1c = [wp.tile([P, DO, nh * P], BF16, tag=f"w1_{k}", name=f"w1_{k}")
                   for k, (h0, nh) in enumerate(w1_chunks)]
            w2c = [wp.tile([P, HC, O], BF16, tag=f"w2_{k}", name=f"w2_{k}") for k in range(NWC)]
            b1_sb = wp.tile([P, HO], F32, tag="b1")
            nc.scalar.dma_start(b1_sb[:], b1[:])
            for k, (h0, nh) in enumerate(w1_chunks):
                nc.sync.dma_start(w1c[k][:], w1_r[:, :, h0 * P:(h0 + nh) * P])
            b2_sb = wp.tile([P, O], F32, tag="b2")
            nc.scalar.dma_start(b2_sb[:], b2[:])
            wt_sb = wp.tile([P, C // P], F32, tag="wt")
            nc.scalar.dma_start(wt_sb[:], wt[:])

            # W2 chunks 1.. are paced behind block-0 relus so they don't
            # race the critical W1 stream during startup
            w2_load_after = {
                4 * k: [(w2c[k], w2_r[:, k * HC:(k + 1) * HC])]
                for k in range(1, NWC)
            }
            w2_load_after[0] = [(w2c[0], w2_r[:, 0:HC])]
            for blk, (n0, cb) in enumerate(starts):
                if blk == 0:
                    x_sb = x0_sb[:, :, :cb]
                else:
                    x_sb = xp.tile([P, DO, CB], BF16, tag="x", name="x_sb")[:, :, :cb]
                    nc.sync.dma_start(x_sb[:], xT_r[:, :, n0:n0 + cb])
                hgs = [hp.tile([P, HG, CB], BF16, tag=f"h{g}", name=f"h{g}")[:, :, :cb]
                       for g in range(NHG)]
                for hi in range(HO):
                    ph = hps.tile([P, CB], F32, tag="ph", name="ph")[:, :cb]
                    ci, off = w1_of_hi[hi]
                    for di in range(DO):
                        nc.tensor.matmul(
                            ph[:],
                            w1c[ci][:, di, off * P:(off + 1) * P],
                            x_sb[:, di],
                            start=(di == 0),
                            stop=(di == DO - 1),
                        )
                    act = nc.scalar.activation(
                        hgs[hi // HG][:, hi % HG], ph[:],
                        mybir.ActivationFunctionType.Relu,
                        bias=b1_sb[:, hi:hi + 1],
                    )
                    if blk == 0 and hi in w2_load_after:
                        # W2 chunk k streams only after L1 consumed W1 chunk
                        # k, so it never races the critical W1 delivery
                        for w2t, w2src in w2_load_after[hi]:
                            dma = nc.scalar.dma_start(w2t[:], w2src)
                            tile.add_dep_helper(
                                dma.ins, act.ins,
                                reason="pace late load behind W1 consumption",
                            )
                for ct in range(cb // P):
                    # hi outer / ot inner: both ot matmuls share the same
                    # stationary hT slice, halving LDWEIGHTS pressure
                    yps_ct = [yps.tile([P, 512], F32, tag="yp", name="yp")
                              for _ in range(OO)]
                    for hi in range(HO):
                        for ot in range(OO):
                            nc.tensor.matmul(
                                yps_ct[ot][:],
                                hgs[hi // HG][:, hi % HG, ct * P:(ct + 1) * P],
                                w2c[hi // HC][:, hi % HC, ot * 512:(ot + 1) * 512],
                                start=(hi == 0),
                                stop=(hi == HO - 1),
                            )
                    for ot in range(OO):
                        o_sb = op.tile([P, 512], F32, tag="o")
                        nc.vector.tensor_add(
                            o_sb[:], yps_ct[ot][:], b2_sb[:, ot * 512:(ot + 1) * 512]
                        )
                        n_idx = n0 // P + ct
                        nc.vector.tensor_scalar_mul(
                            o_sb[:], o_sb[:], wt_sb[:, n_idx:n_idx + 1]
                        )
                        nc.sync.dma_start(
                            y_r[:, n_idx, ot * 512:(ot + 1) * 512], o_sb[:]
                        )
    nc.finalize()
    return nc


def kernel(x, W1, b1, W2, b2, gate_w, gate_b):
    x = np.ascontiguousarray(x, dtype=np.float32)
    W1 = np.asarray(W1, dtype=np.float32)
    b1 = np.asarray(b1, dtype=np.float32)
    W2 = np.asarray(W2, dtype=np.float32)
    b2 = np.asarray(b2, dtype=np.float32)
    gate_w = np.ascontiguousarray(gate_w, dtype=np.float32)
    gate_b = np.asarray(gate_b, dtype=np.float32)

    B, D = x.shape
    E, _, H = W1.shape
    O = W2.shape[2]
    assert E == N_CORES and B % (N_CORES * 512) == 0 and D % P == 0
    core_ids = list(range(N_CORES))

    # ---- Gate logits on host (the routing / sharding decision) ----
    logits = x @ gate_w + gate_b[None, :]

    # ---- Host: top-2 routing (the expert-parallel sharding decision) ----
    lg = logits.astype(np.float64)
    lg -= lg.max(axis=1, keepdims=True)
    probs = np.exp(lg)
    probs /= probs.sum(axis=1, keepdims=True)
    order = np.argsort(-probs, axis=1, kind="stable")[:, :2]
    p_top = np.take_along_axis(probs, order, axis=1)
    w_top = p_top / p_top.sum(axis=1, keepdims=True)  # [B, 2]

    idx_e, wt_e = [], []
    for e in range(E):
        m0 = order[:, 0] == e
        m1 = order[:, 1] == e
        sel = m0 | m1
        idx = np.nonzero(sel)[0]
        w = np.where(m0[sel], w_top[sel, 0], w_top[sel, 1]).astype(np.float32)
        idx_e.append(idx)
        wt_e.append(w)
    max_count = max(len(i) for i in idx_e)
    C = max(CB, ((max_count + P - 1) // P) * P)

    # ---- Phase 2: expert FFN on device (expert-parallel) ----
    key = ("expert", D, H, O, C)
    if key not in _build_cache:
        _build_cache[key] = _build_expert(D, H, O, C)
    nc_exp = _build_cache[key]

    in_maps = []
    for e in range(E):
        n_e = len(idx_e[e])
        xT_pad = np.zeros((D, C), dtype=_bf16_np)
        xT_pad[:, :n_e] = x[idx_e[e]].T.astype(_bf16_np)
        wt_pad = np.zeros(C, dtype=np.float32)
        wt_pad[:n_e] = wt_e[e]
        in_maps.append({
            "xT": xT_pad,
            "w1": W1[e].astype(_bf16_np),
            "w2": W2[e].astype(_bf16_np),
            "b1": np.ascontiguousarray(b1[e].reshape(H // P, P).T),
            "b2": np.ascontiguousarray(np.broadcast_to(b2[e], (P, O))),
            "wt": np.ascontiguousarray(wt_pad.reshape(C // P, P).T),
        })
    res = run_bass_kernel_spmd(nc_exp, in_maps, core_ids=core_ids)

    # ---- Host: un-permute and combine the two expert contributions ----
    out = np.zeros((B, O), dtype=np.float32)
    for e in range(E):
        n_e = len(idx_e[e])
        if n_e:
            out[idx_e[e]] += res.results[e]["y"][:n_e]
    return out



# revision 9
# speedup vs baseline: 1.0040x; 1.0040x over previous
"""MoE layer (top-2 of 8 experts) on 8 TRN2 NeuronCores.

Strategy:
  Host: gate logits + softmax + top-2 + renormalized weights (the
      routing / sharding decision), build per-expert token index lists,
      pad to a common capacity C (multiple of the token block).
  Device (expert-parallel): core e runs its expert's FFN over
      the tokens routed to it: y = (relu(x@W1+b1)@W2 + b2) * w_token.
      bf16 matmuls, fp32 PSUM accumulation, weights SBUF-resident.
  Host: scatter-add the two scaled contributions per token.
"""

import numpy as np
import ml_dtypes

import concourse.mybir as mybir
import concourse.tile as tile
from concourse import bacc
from concourse.bass_utils import run_bass_kernel_spmd

P = 128
N_CORES = 8
CB = 384  # phase-2 token block
BF16 = mybir.dt.bfloat16
F32 = mybir.dt.float32
_bf16_np = ml_dtypes.bfloat16

_build_cache = {}


def _build_expert(D, H, O, C):
    """Per-core expert FFN over C (padded) routed tokens.

    y[C, O] = (relu(x @ W1 + b1) @ W2 + b2) * w_token[:, None]
    computed as hT = W1.T-slices @ xT (keeps H on partitions), then
    y = hT-slices.T @ W2 (tokens back on partitions). No transposes on
    device: xT / b1 / wt come host-prearranged.
    """
    nc = bacc.Bacc(None, target_bir_lowering=False)
    xT = nc.dram_tensor("xT", [D, C], BF16, kind="ExternalInput")
    w1 = nc.dram_tensor("w1", [D, H], BF16, kind="ExternalInput")
    w2 = nc.dram_tensor("w2", [H, O], BF16, kind="ExternalInput")
    b1 = nc.dram_tensor("b1", [P, H // P], F32, kind="ExternalInput")
    b2 = nc.dram_tensor("b2", [P, O], F32, kind="ExternalInput")
    wt = nc.dram_tensor("wt", [P, C // P], F32, kind="ExternalInput")
    y = nc.dram_tensor("y", [C, O], F32, kind="ExternalOutput")
    DO, HO = D // P, H // P
    OO = O // 512
    # token blocks of CB, trailing 128-block if C % CB != 0
    starts = []
    pos = 0
    while pos < C:
        cb = CB if C - pos >= CB else C - pos
        starts.append((pos, cb))
        pos += cb
    # chunk the weight loads so the first matmuls start after ~1MB of DMA;
    # the first two W1 chunks are half-size so L1 starts even earlier
    HC = 4                   # h-tiles (of 128) per W2 weight chunk
    NWC = HO // HC           # number of W2 weight chunks
    w1_chunks = [(0, 1), (1, 1), (2, 2)] + [(h, 4) for h in range(4, HO, 4)]
    w1_of_hi = {}            # hi -> (chunk index, offset within chunk)
    for ci, (h0, nh) in enumerate(w1_chunks):
        for j in range(nh):
            w1_of_hi[h0 + j] = (ci, j)
    HG = 8                   # h-tiles per hT group tile (finer L2 deps)
    NHG = HO // HG
    y_r = y.rearrange("(n p) o -> p n o", p=P)
    w1_r = w1.rearrange("(do p) h -> p do h", p=P)
    w2_r = w2.rearrange("(ho p) o -> p ho o", p=P)
    with tile.TileContext(nc) as tc:
        with (
            tc.tile_pool(name="wpool", bufs=1) as wp,
            tc.tile_pool(name="xpool", bufs=3) as xp,
            tc.tile_pool(name="hpool", bufs=2) as hp,
            tc.tile_pool(name="opool", bufs=4) as op,
            tc.tile_pool(name="hps", bufs=4, space="PSUM") as hps,
            tc.tile_pool(name="yps", bufs=3, space="PSUM") as yps,
        ):
            xT_r = xT.rearrange("(do p) c -> p do c", p=P)
            # startup-critical DMAs: W1 chunks split across the sync AND
            # scalar rings (doubles early W1 bandwidth; L1 of block 0
            # chases W1's delivery); block-0 x leads on sync, b1 on
            # scalar, W2/b2/wt and later x blocks ride the gpsimd ring.
            x0_sb = xp.tile([P, DO, CB], BF16, tag="x")
            for dj in range(0, DO, 4):
                nc.scalar.dma_start(
                    x0_sb[:, dj:dj + 4, :starts[0][1]],
                    xT_r[:, dj:dj + 4, 0:starts[0][1]],
                )
            w1c = [wp.tile([P, DO, nh * P], BF16, tag=f"w1_{k}", name=f"w1_{k}")
                   for k, (h0, nh) in enumerate(w1_chunks)]
            w2c = [wp.tile([P, HC, O], BF16, tag=f"w2_{k}", name=f"w2_{k}") for k in range(NWC)]
            b1_sb = wp.tile([P, HO], F32, tag="b1")
            nc.scalar.dma_start(b1_sb[:], b1[:])
            for k, (h0, nh) in enumerate(w1_chunks):
                nc.sync.dma_start(w1c[k][:], w1_r[:, :, h0 * P:(h0 + nh) * P])
            b2_sb = wp.tile([P, O], F32, tag="b2")
            nc.scalar.dma_start(b2_sb[:], b2[:])
            wt_sb = wp.tile([P, C // P], F32, tag="wt")
            nc.scalar.dma_start(wt_sb[:], wt[:])

            # W2 chunks 1.. are paced behind block-0 relus so they don't
            # race the critical W1 stream during startup
            # W2 chunk k streams behind act hi=16+k: W1's 8MB has fully
            # landed by then, so W2 never competes with the W1 chase, yet
            # every chunk still arrives well before block-0's y-matmuls.
            w2_load_after = {
                16 + k: [(w2c[k], w2_r[:, k * HC:(k + 1) * HC])]
                for k in range(NWC)
            }
            for blk, (n0, cb) in enumerate(starts):
                if blk == 0:
                    x_sb = x0_sb[:, :, :cb]
                else:
                    x_sb = xp.tile([P, DO, CB], BF16, tag="x", name="x_sb")[:, :, :cb]
                    nc.sync.dma_start(x_sb[:], xT_r[:, :, n0:n0 + cb])
                hgs = [hp.tile([P, HG, CB], BF16, tag=f"h{g}", name=f"h{g}")[:, :, :cb]
                       for g in range(NHG)]
                for hi in range(HO):
                    ph = hps.tile([P, CB], F32, tag="ph", name="ph")[:, :cb]
                    ci, off = w1_of_hi[hi]
                    for di in range(DO):
                        nc.tensor.matmul(
                            ph[:],
                            w1c[ci][:, di, off * P:(off + 1) * P],
                            x_sb[:, di],
                            start=(di == 0),
                            stop=(di == DO - 1),
                        )
                    act = nc.scalar.activation(
                        hgs[hi // HG][:, hi % HG], ph[:],
                        mybir.ActivationFunctionType.Relu,
                        bias=b1_sb[:, hi:hi + 1],
                    )
                    if blk == 0 and hi in w2_load_after:
                        # W2 chunk k streams only after L1 consumed W1 chunk
                        # k, so it never races the critical W1 delivery
                        for w2t, w2src in w2_load_after[hi]:
                            dma = nc.scalar.dma_start(w2t[:], w2src)
                            tile.add_dep_helper(
                                dma.ins, act.ins,
                                reason="pace late load behind W1 consumption",
                            )
                for ct in range(cb // P):
                    # hi outer / ot inner: both ot matmuls share the same
                    # stationary hT slice, halving LDWEIGHTS pressure
                    yps_ct = [yps.tile([P, 512], F32, tag="yp", name="yp")
                              for _ in range(OO)]
                    for hi in range(HO):
                        for ot in range(OO):
                            nc.tensor.matmul(
                                yps_ct[ot][:],
                                hgs[hi // HG][:, hi % HG, ct * P:(ct + 1) * P],
                                w2c[hi // HC][:, hi % HC, ot * 512:(ot + 1) * 512],
                                start=(hi == 0),
                                stop=(hi == HO - 1),
                            )
                    for ot in range(OO):
                        o_sb = op.tile([P, 512], F32, tag="o")
                        nc.vector.tensor_add(
                            o_sb[:], yps_ct[ot][:], b2_sb[:, ot * 512:(ot + 1) * 512]
                        )
                        n_idx = n0 // P + ct
                        nc.vector.tensor_scalar_mul(
                            o_sb[:], o_sb[:], wt_sb[:, n_idx:n_idx + 1]
                        )
                        nc.sync.dma_start(
                            y_r[:, n_idx, ot * 512:(ot + 1) * 512], o_sb[:]
                        )
    nc.finalize()
    return nc


def kernel(x, W1, b1, W2, b2, gate_w, gate_b):
    x = np.ascontiguousarray(x, dtype=np.float32)
    W1 = np.asarray(W1, dtype=np.float32)
    b1 = np.asarray(b1, dtype=np.float32)
    W2 = np.asarray(W2, dtype=np.float32)
    b2 = np.asarray(b2, dtype=np.float32)
    gate_w = np.ascontiguousarray(gate_w, dtype=np.float32)
    gate_b = np.asarray(gate_b, dtype=np.float32)

    B, D = x.shape
    E, _, H = W1.shape
    O = W2.shape[2]
    assert E == N_CORES and B % (N_CORES * 512) == 0 and D % P == 0
    core_ids = list(range(N_CORES))

    # ---- Gate logits on host (the routing / sharding decision) ----
    logits = x @ gate_w + gate_b[None, :]

    # ---- Host: top-2 routing (the expert-parallel sharding decision) ----
    lg = logits.astype(np.float64)
    lg -= lg.max(axis=1, keepdims=True)
    probs = np.exp(lg)
    probs /= probs.sum(axis=1, keepdims=True)
    order = np.argsort(-probs, axis=1, kind="stable")[:, :2]
    p_top = np.take_along_axis(probs, order, axis=1)
    w_top = p_top / p_top.sum(axis=1, keepdims=True)  # [B, 2]

    idx_e, wt_e = [], []
    for e in range(E):
        m0 = order[:, 0] == e
        m1 = order[:, 1] == e
        sel = m0 | m1
        idx = np.nonzero(sel)[0]
        w = np.where(m0[sel], w_top[sel, 0], w_top[sel, 1]).astype(np.float32)
        idx_e.append(idx)
        wt_e.append(w)
    max_count = max(len(i) for i in idx_e)
    C = max(CB, ((max_count + P - 1) // P) * P)

    # ---- Phase 2: expert FFN on device (expert-parallel) ----
    key = ("expert", D, H, O, C)
    if key not in _build_cache:
        _build_cache[key] = _build_expert(D, H, O, C)
    nc_exp = _build_cache[key]

    in_maps = []
    for e in range(E):
        n_e = len(idx_e[e])
        xT_pad = np.zeros((D, C), dtype=_bf16_np)
        xT_pad[:, :n_e] = x[idx_e[e]].T.astype(_bf16_np)
        wt_pad = np.zeros(C, dtype=np.float32)
        wt_pad[:n_e] = wt_e[e]
        in_maps.append({
            "xT": xT_pad,
            "w1": W1[e].astype(_bf16_np),
            "w2": W2[e].astype(_bf16_np),
            "b1": np.ascontiguousarray(b1[e].reshape(H // P, P).T),
            "b2": np.ascontiguousarray(np.broadcast_to(b2[e], (P, O))),
            "wt": np.ascontiguousarray(wt_pad.reshape(C // P, P).T),
        })
    res = run_bass_kernel_spmd(nc_exp, in_maps, core_ids=core_ids)

    # ---- Host: un-permute and combine the two expert contributions ----
    out = np.zeros((B, O), dtype=np.float32)
    for e in range(E):
        n_e = len(idx_e[e])
        if n_e:
            out[idx_e[e]] += res.results[e]["y"][:n_e]
    return out



# revision 12
# speedup vs baseline: 1.0458x; 1.0416x over previous
"""MoE layer (top-2 of 8 experts) on 8 TRN2 NeuronCores.

Strategy:
  Host: gate logits + softmax + top-2 + renormalized weights (the
      routing / sharding decision), build per-expert token index lists.
  Device (paired-expert parallel): experts are paired big-with-small
      into 4 groups; the two cores of group g each run HALF of each
      paired expert's tokens (two segments per core, token-exact chunk
      widths -- no 128-padding). Per chunk: hT = W1-slices.T @ xT
      (H on partitions), relu via activation, then the TRANSPOSED
      second matmul yT = W2-slab.T @ hT (O on partitions, tokens on
      the free dim) so compute scales with the exact token count.
      Segment-A W1 is SBUF-resident; segment-B W1 overwrites it via
      pool WAR deps during the last A chunk; W2 slabs stream per chunk.
  Host: scale columns by the gate weight, transpose, scatter-add.
"""

import numpy as np
import ml_dtypes

import concourse.mybir as mybir
import concourse.tile as tile
from concourse import bacc
from concourse.bass_utils import run_bass_kernel_spmd

P = 128
N_CORES = 8
BF16 = mybir.dt.bfloat16
F32 = mybir.dt.float32
_bf16_np = ml_dtypes.bfloat16

_build_cache = {}


def _chunks_of(T):
    """Near-equal chunks <= 512 wide; >= ~200 so LDWEIGHTS stays hidden."""
    n = -(-T // 512)
    base = T // n
    rem = T - base * n
    out = [base + (1 if i < rem else 0) for i in range(n)]
    assert all(c >= 200 for c in out), out
    return out


def _build_expert2(D, H, O, segs):
    """Two-expert core: segs = (chunk widths of segment A, of segment B).

    Inputs: xc [D, TA+TB] bf16 column-packed tokens; w1a/w1b [D, H] bf16;
    w2a/w2b [O/P, P, H/P, P] bf16 slab-major (slab[ot][p_h][kt][oc] =
    W2[kt*P+p_h, ot*P+oc]); b1a/b1b [P, H/P] f32; b2a/b2b [P, O/P] f32.
    Output: yT [O, TA+TB] f32 = (relu(x W1 + b1) W2 + b2).T, unscaled.
    """
    nc = bacc.Bacc(None, target_bir_lowering=False)
    TA, TB = sum(segs[0]), sum(segs[1])
    TT = TA + TB
    xc = nc.dram_tensor("xc", [D, TT], BF16, kind="ExternalInput")
    w1_d = [nc.dram_tensor(n, [D, H], BF16, kind="ExternalInput")
            for n in ("w1a", "w1b")]
    w2_d = [nc.dram_tensor(n, [O, H], BF16, kind="ExternalInput")
            for n in ("w2a", "w2b")]
    b1_d = [nc.dram_tensor(n, [P, H // P], F32, kind="ExternalInput")
            for n in ("b1a", "b1b")]
    b2_d = [nc.dram_tensor(n, [P, O // P], F32, kind="ExternalInput")
            for n in ("b2a", "b2b")]
    yT = nc.dram_tensor("yT", [O, TT], F32, kind="ExternalOutput")

    DO, HO, OT = D // P, H // P, O // P
    HG = 8
    NHG = HO // HG
    CW = 512
    xc_r = xc.rearrange("(do p) t -> p do t", p=P)
    w2_rs = [w.rearrange("(ot p) c -> p ot c", p=P) for w in w2_d]
    yT_r = yT.rearrange("(ot p) t -> p ot t", p=P)
    w1_rs = [w.rearrange("(do p) h -> p do h", p=P) for w in w1_d]

    w1_chunks = [(0, 1), (1, 1), (2, 2)] + [(h, 4) for h in range(4, HO, 4)]
    w1_of_hi = {}
    for ci, (h0, nh) in enumerate(w1_chunks):
        for j in range(nh):
            w1_of_hi[h0 + j] = (ci, j)
    NW1 = len(w1_chunks)

    with tile.TileContext(nc) as tc:
        with (
            tc.tile_pool(name="w1pool", bufs=1) as w1p,
            tc.tile_pool(name="w2pool", bufs=3) as w2p,
            tc.tile_pool(name="cpool", bufs=1) as cp,
            tc.tile_pool(name="xpool", bufs=2) as xp,
            tc.tile_pool(name="hpool", bufs=2) as hp,
            tc.tile_pool(name="opool", bufs=4) as op,
            tc.tile_pool(name="hps", bufs=4, space="PSUM") as hps,
            tc.tile_pool(name="yps", bufs=3, space="PSUM") as yps,
        ):
            x0_sb = xp.tile([P, DO, CW], BF16, tag="x", name="x0_sb")
            w0 = segs[0][0]
            for dj in range(0, DO, 4):
                nc.scalar.dma_start(x0_sb[:, dj:dj + 4, :w0],
                                    xc_r[:, dj:dj + 4, 0:w0])
            b1_sb = [cp.tile([P, HO], F32, tag=f"b1{s}", name=f"b1{s}")
                     for s in range(2)]
            nc.scalar.dma_start(b1_sb[0][:], b1_d[0][:])
            w1t = {0: [w1p.tile([P, DO, nh * P], BF16, tag=f"w1_{k}",
                               name=f"w1a_{k}")
                       for k, (h0, nh) in enumerate(w1_chunks)]}
            for k, (h0, nh) in enumerate(w1_chunks):
                nc.sync.dma_start(w1t[0][k][:],
                                  w1_rs[0][:, :, h0 * P:(h0 + nh) * P])
            nc.scalar.dma_start(b1_sb[1][:], b1_d[1][:])
            b2_sb = [cp.tile([P, OT], F32, tag=f"b2{s}", name=f"b2{s}")
                     for s in range(2)]
            nc.scalar.dma_start(b2_sb[0][:], b2_d[0][:])
            nc.scalar.dma_start(b2_sb[1][:], b2_d[1][:])

            chunks = []
            pos = 0
            for s in (0, 1):
                for w in segs[s]:
                    chunks.append((s, pos, w))
                    pos += w
            lastA = len(segs[0]) - 1

            def emit_w1b(k0, k1):
                # gen-2 tiles share w1a's addresses; pool WAR deps hold
                # each load until the last seg-A reader of that chunk
                for k in range(k0, k1):
                    h0, nh = w1_chunks[k]
                    t = w1p.tile([P, DO, nh * P], BF16, tag=f"w1_{k}",
                                 name=f"w1b_{k}")
                    nc.sync.dma_start(t[:], w1_rs[1][:, :, h0 * P:(h0 + nh) * P])
                    w1t.setdefault(1, []).append(t)

            for cidx, (s, c0, w) in enumerate(chunks):
                if cidx == 0:
                    x_sb = x0_sb[:, :, :w]
                else:
                    x_sb = xp.tile([P, DO, CW], BF16, tag="x",
                                   name="x_sb")[:, :, :w]
                    nc.sync.dma_start(x_sb[:], xc_r[:, :, c0:c0 + w])
                hgs = [hp.tile([P, HG, CW], BF16, tag=f"h{g}",
                               name=f"h{g}")[:, :, :w] for g in range(NHG)]
                acts = []
                # ---- mm1: h[hi] = relu(W1[:, hi].T @ x + b1) ----
                for hi in range(HO):
                    ph = hps.tile([P, CW], F32, tag="ph", name="ph")[:, :w]
                    ci, off = w1_of_hi[hi]
                    for di in range(DO):
                        nc.tensor.matmul(
                            ph[:],
                            w1t[s][ci][:, di, off * P:(off + 1) * P],
                            x_sb[:, di],
                            start=(di == 0),
                            stop=(di == DO - 1),
                        )
                    acts.append(nc.scalar.activation(
                        hgs[hi // HG][:, hi % HG], ph[:],
                        mybir.ActivationFunctionType.Relu,
                        bias=b1_sb[s][:, hi:hi + 1],
                    ))
                if cidx == lastA:
                    emit_w1b(0, 6)
                # ---- mm2 (transposed): yT[ot] = W2-slab[ot].T @ h ----
                for ot in range(OT):
                    slab = w2p.tile([P, HO, P], BF16, tag="w2s", name="w2s")
                    dma = nc.scalar.dma_start(
                        slab[:],
                        w2_rs[s][:, ot].rearrange("p (ho oc) -> p ho oc", oc=P),
                    )
                    if cidx == 0:
                        # keep chunk-0 slabs off the W1a chase: start them
                        # only once mm1 of chunk 0 is well underway
                        tile.add_dep_helper(
                            dma.ins, acts[16 + 2 * ot].ins,
                            reason="pace first-chunk W2 slabs behind W1a",
                        )
                    yp = yps.tile([P, CW], F32, tag="yp", name="yp")[:, :w]
                    for kt in range(HO):
                        nc.tensor.matmul(
                            yp[:],
                            slab[:, kt],
                            hgs[kt // HG][:, kt % HG],
                            start=(kt == 0),
                            stop=(kt == HO - 1),
                        )
                    o_sb = op.tile([P, CW], F32, tag="o", name="o_sb")[:, :w]
                    nc.vector.tensor_scalar_add(
                        o_sb[:], yp[:], b2_sb[s][:, ot:ot + 1])
                    nc.sync.dma_start(yT_r[:, ot, c0:c0 + w], o_sb[:])
                if cidx == lastA:
                    emit_w1b(6, NW1)
    nc.finalize()
    return nc


def kernel(x, W1, b1, W2, b2, gate_w, gate_b):
    x = np.ascontiguousarray(x, dtype=np.float32)
    W1 = np.asarray(W1, dtype=np.float32)
    b1 = np.asarray(b1, dtype=np.float32)
    W2 = np.asarray(W2, dtype=np.float32)
    b2 = np.asarray(b2, dtype=np.float32)
    gate_w = np.ascontiguousarray(gate_w, dtype=np.float32)
    gate_b = np.asarray(gate_b, dtype=np.float32)

    B, D = x.shape
    E, _, H = W1.shape
    O = W2.shape[2]
    assert E == N_CORES and D % P == 0 and H % P == 0 and O % P == 0
    core_ids = list(range(N_CORES))

    # ---- Gate + routing on host (the sharding decision) ----
    logits = x @ gate_w + gate_b[None, :]
    lg = logits.astype(np.float64)
    lg -= lg.max(axis=1, keepdims=True)
    probs = np.exp(lg)
    probs /= probs.sum(axis=1, keepdims=True)
    order = np.argsort(-probs, axis=1, kind="stable")[:, :2]
    p_top = np.take_along_axis(probs, order, axis=1)
    w_top = (p_top / p_top.sum(axis=1, keepdims=True)).astype(np.float32)

    idx_e, wt_e = [], []
    for e in range(E):
        m0 = order[:, 0] == e
        m1 = order[:, 1] == e
        sel = m0 | m1
        idx = np.nonzero(sel)[0]
        wt = np.where(m0[sel], w_top[sel, 0], w_top[sel, 1]).astype(np.float32)
        idx_e.append(idx)
        wt_e.append(wt)
    counts = np.array([len(i) for i in idx_e])

    # ---- Pair experts big-with-small into 4 groups of 2 cores ----
    order_e = np.argsort(-counts)
    pairs = [(order_e[i], order_e[E - 1 - i]) for i in range(E // 2)]
    TA = int(max(-(-counts[a] // 2) for a, _ in pairs))
    TB = int(max(-(-counts[b] // 2) for _, b in pairs))
    segs = (tuple(_chunks_of(TA)), tuple(_chunks_of(TB)))

    key = ("expert2", D, H, O, segs)
    if key not in _build_cache:
        _build_cache[key] = _build_expert2(D, H, O, segs)
    nc = _build_cache[key]

    def w2_slabs(e):
        s = W2[e].reshape(H // P, P, O // P, P).transpose(2, 1, 0, 3)
        return np.ascontiguousarray(s.reshape(O, H)).astype(_bf16_np)

    xT_bf = np.ascontiguousarray(x.T).astype(_bf16_np)  # [D, B]
    in_maps, slices = [], []
    for g, (a, be) in enumerate(pairs):
        ia, ib = idx_e[a], idx_e[be]
        ha, hb = -(-len(ia) // 2), -(-len(ib) // 2)
        w1a = W1[a].astype(_bf16_np)
        w1b = W1[be].astype(_bf16_np)
        w2a, w2b = w2_slabs(a), w2_slabs(be)
        b1a = np.ascontiguousarray(b1[a].reshape(H // P, P).T)
        b1b = np.ascontiguousarray(b1[be].reshape(H // P, P).T)
        b2a = np.ascontiguousarray(b2[a].reshape(O // P, P).T)
        b2b = np.ascontiguousarray(b2[be].reshape(O // P, P).T)
        for half in range(2):
            sa = ia[half * ha:(half + 1) * ha]
            sb = ib[half * hb:(half + 1) * hb]
            xcm = np.zeros((D, TA + TB), dtype=_bf16_np)
            xcm[:, :len(sa)] = xT_bf[:, sa]
            xcm[:, TA:TA + len(sb)] = xT_bf[:, sb]
            in_maps.append({
                "xc": xcm, "w1a": w1a, "w1b": w1b, "w2a": w2a, "w2b": w2b,
                "b1a": b1a, "b1b": b1b, "b2a": b2a, "b2b": b2b,
            })
            wa = wt_e[a][half * ha:(half + 1) * ha]
            wb = wt_e[be][half * hb:(half + 1) * hb]
            slices.append((sa, wa, sb, wb))
    res = run_bass_kernel_spmd(nc, in_maps, core_ids=core_ids)

    # ---- Host: gate-weight scale, transpose, scatter-add ----
    out = np.zeros((B, O), dtype=np.float32)
    for c, (sa, wa, sb, wb) in enumerate(slices):
        yTo = res.results[c]["yT"]  # [O, TA+TB] f32
        if len(sa):
            out[sa] += yTo[:, :len(sa)].T * wa[:, None]
        if len(sb):
            out[sb] += yTo[:, TA:TA + len(sb)].T * wb[:, None]
    return out
